# revision 12
# baseline (speedup 1.0000x reference)
"""Causal self-attention on 8 Trainium2 NeuronCores.

Reference (fp32):
    qkv = x @ W_qkv + b_qkv ; split q,k,v ; heads H=16, Dh=64
    scores = q @ k^T / sqrt(Dh), causal mask, softmax
    out = (attn @ v) re-merged ; y = out @ W_proj + b_proj

Sharding: tensor-parallel over heads x data-parallel over batch.
Core c (0..7) owns batch b = c//4 and head group g = c%4 (heads 4g..4g+3).
Each core computes q^T,k^T,v for its 4 heads from x[b]^T, runs causal
attention (scores transposed layout, exp without max-subtraction -- scores
are O(5) so fp32 exp is safe, denominator via an appended ones-column in
the V matmul).

v2 changes vs the ReduceScatter baseline:
  - per-512-column pipeline: qkv chunk j is emitted together with
    attention chunk j so the tensor engine never drains between phases.
  - combine across the 4 cores of a batch via fp16 AllGather of the
    normalized O^T (256 KiB in / 1 MiB out per chunk) instead of fp32
    ReduceScatter of partial y (2 MiB per chunk); each core then runs the
    full-contraction projection for its 256 output channels.
  - exp batched: both heads of a pair share one [128, 1024] PSUM score
    tile and a single Exp activation (halves ACT instruction overhead).
  - ScalarE carries only the exp stream; all DMA issues live on the
    sync/gpsimd queues.

Matmuls run in fp16 (full PE speed + fast weight load, ~3e-4 rel err
end to end vs the fp32 reference).
"""

import numpy as np

import concourse.bacc as bacc
import concourse.mybir as mybir
import concourse.tile as tile
from concourse.bass_utils import run_bass_kernel_spmd

B = 2
T = 2048
C = 1024
H = 16
DH = 64
G = 4  # heads per core
N_CORES = 8
TQ = 512  # q-chunk width
NKT = T // 128  # k tiles per head
NJQ = T // TQ  # q chunks
NCK = C // 128  # contraction tiles over model dim
SCALE = 1.0 / np.sqrt(DH)
GROUPS = [[0, 1, 2, 3], [4, 5, 6, 7]]

F32 = mybir.dt.float32
F32R = mybir.dt.float32r
BF16 = mybir.dt.bfloat16
FP16 = mybir.dt.float16
ATT_DT = FP16
MM_DT = FP16

_PROG = None


def _build_program():
    nc = bacc.Bacc(
        "TRN2", target_bir_lowering=False, debug=False, num_devices=N_CORES
    )
    xt_d = nc.dram_tensor("xt", [C, T], MM_DT, kind="ExternalInput").ap()
    wq_d = nc.dram_tensor("wq", [C, G * DH], MM_DT, kind="ExternalInput").ap()
    wk_d = nc.dram_tensor("wk", [C, G * DH], MM_DT, kind="ExternalInput").ap()
    wv_d = nc.dram_tensor("wv", [C, G * DH], MM_DT, kind="ExternalInput").ap()
    # W_proj[:, 256g:256g+256] -- full contraction dim, this core's out cols
    wp_d = nc.dram_tensor("wp", [C, C // 4], MM_DT, kind="ExternalInput").ap()
    bq_d = nc.dram_tensor("bq", [G * DH, 1], F32, kind="ExternalInput").ap()
    bk_d = nc.dram_tensor("bk", [G * DH, 1], F32, kind="ExternalInput").ap()
    bv_d = nc.dram_tensor("bv", [1, G * DH], F32, kind="ExternalInput").ap()
    bp_d = nc.dram_tensor("bp", [C // 4, 1], F32, kind="ExternalInput").ap()
    # maskw[k, h*512 + u] = (u >= k), h = 0,1 -- one mul covers both heads
    mask_d = nc.dram_tensor("mask", [128, 1024], ATT_DT, kind="ExternalInput").ap()
    ones_d = nc.dram_tensor("ones", [128, 64], F32R, kind="ExternalInput").ap()
    onesb_d = nc.dram_tensor("onesb", [128, 64], ATT_DT, kind="ExternalInput").ap()
    ag_in = [
        nc.dram_tensor(f"ag_in{j}", [C // 4, TQ], ATT_DT).ap() for j in range(NJQ)
    ]
    ag_out = [
        nc.dram_tensor(f"ag_out{j}", [C, TQ], ATT_DT).ap() for j in range(NJQ)
    ]
    y_d = nc.dram_tensor("y", [C // 4, T], F32, kind="ExternalOutput").ap()

    with tile.TileContext(nc) as tc:
        with (
            nc.allow_low_precision(reason="fp16 matmul pipeline by design"),
            tc.tile_pool(name="ll", bufs=1) as ll,
            tc.tile_pool(name="rp", bufs=4) as rpp,
            tc.tile_pool(name="es", bufs=4) as esp,
            tc.tile_pool(name="oc", bufs=4) as ocp,
            tc.tile_pool(name="og", bufs=2) as ogp,
            tc.tile_pool(name="ps0", bufs=2, space="PSUM") as sp0,
            tc.tile_pool(name="ov", bufs=2, space="PSUM") as ovp,
            tc.tile_pool(name="px", bufs=2, space="PSUM") as pxp,
        ):
            # ---- long-lived tiles -------------------------------------
            qT = [ll.tile([128, T], ATT_DT, tag=f"qT{p}", name=f"qT{p}") for p in range(2)]
            kT = [ll.tile([128, T], ATT_DT, tag=f"kT{p}", name=f"kT{p}") for p in range(2)]
            oT = [ll.tile([128, T], ATT_DT, tag=f"oT{p}", name=f"oT{p}") for p in range(2)]
            vaug = [ll.tile([128, G * 65], ATT_DT, tag=f"va{t}", name=f"va{t}") for t in range(NKT)]

            mask = ll.tile([128, 1024], ATT_DT, tag="mask")
            nc.sync.dma_start(out=mask[:], in_=mask_d[:])
            ones_sb = ll.tile([128, 64], F32R, tag="ones")
            nc.sync.dma_start(out=ones_sb[:], in_=ones_d[:])
            bq_sb = [ll.tile([128, 1], F32, tag=f"bq{p}", name=f"bq{p}") for p in range(2)]
            bk_sb = [ll.tile([128, 1], F32, tag=f"bk{p}", name=f"bk{p}") for p in range(2)]
            for p in range(2):
                nc.sync.dma_start(
                    out=bq_sb[p][:], in_=bq_d[p * 128 : (p + 1) * 128, :]
                )
                nc.sync.dma_start(
                    out=bk_sb[p][:], in_=bk_d[p * 128 : (p + 1) * 128, :]
                )
            bv_sb = ll.tile([1, G * DH], F32, tag="bv")
            nc.sync.dma_start(out=bv_sb[:], in_=bv_d[:])
            bp_sb = [ll.tile([128, 1], F32, tag=f"bp{i}", name=f"bp{i}") for i in range(2)]
            for i in range(2):
                nc.sync.dma_start(
                    out=bp_sb[i][:], in_=bp_d[i * 128 : (i + 1) * 128, :]
                )
            # projection weights: 8 x [128, 256], loaded late (first use ~40us in)
            wp_sb = [
                ll.tile([128, C // 4], MM_DT, tag=f"wp{k}", name=f"wp{k}")
                for k in range(NCK)
            ]
            for k in range(NCK):
                nc.gpsimd.dma_start(
                    out=wp_sb[k][:], in_=wp_d[k * 128 : (k + 1) * 128, :]
                )

            # bv broadcast across partitions (via ones-row matmul)
            ones_row = ll.tile([1, 128], F32R, tag="ones_row")
            nc.sync.dma_start(out=ones_row[:, 0:64], in_=ones_d[0:1, :])
            nc.sync.dma_start(out=ones_row[:, 64:128], in_=ones_d[0:1, :])
            bv_r = ll.tile([1, G * DH], F32R, tag="bvr")
            nc.vector.tensor_copy(out=bv_r[:], in_=bv_sb[:])
            bvb_ps = pxp.tile([128, TQ], F32, tag="x", name="bvb_ps")
            bvb_sb = ll.tile([128, G * DH], F32, tag="bvb")
            nc.tensor.matmul(
                bvb_ps[:, 0 : G * DH], lhsT=ones_row[:], rhs=bv_r[:], start=True, stop=True
            )
            nc.vector.tensor_copy(out=bvb_sb[:], in_=bvb_ps[:, 0 : G * DH])

            # ---- input / weight streaming -----------------------------
            wq_sb, wk_sb, wv_sb = [], [], []
            xt_sb = [
                ll.tile([128, T], MM_DT, tag=f"xt{k}", name=f"xt{k}")
                for k in range(NCK)
            ]
            # j=0 column of xt + the qkv weights, interleaved k-major on two
            # queues, so the first qkv chains unblock as early as possible
            for k in range(NCK):
                (nc.sync, nc.scalar)[k % 2].dma_start(
                    out=xt_sb[k][:, 0:TQ],
                    in_=xt_d[k * 128 : (k + 1) * 128, 0:TQ],
                )
                for qi, (name, dst, src) in enumerate(
                    (
                        ("q", wq_sb, wq_d),
                        ("k", wk_sb, wk_d),
                        ("v", wv_sb, wv_d),
                    )
                ):
                    t = ll.tile([128, G * DH], MM_DT, tag=f"w{name}{k}", name=f"w{name}{k}")
                    (nc.gpsimd, nc.scalar, nc.gpsimd)[(k + qi) % 3].dma_start(
                        out=t[:], in_=src[k * 128 : (k + 1) * 128, :]
                    )
                    dst.append(t)
            for j in range(1, NJQ):
                for k in range(NCK):
                    eng = (nc.sync, nc.gpsimd)[k % 2]
                    eng.dma_start(
                        out=xt_sb[k][:, j * TQ : (j + 1) * TQ],
                        in_=xt_d[k * 128 : (k + 1) * 128, j * TQ : (j + 1) * TQ],
                    )

            rp_map = {}

            def emit_qkv(j):
                # q^T / k^T chains for this column chunk
                for wsb, bsb, dst in ((wq_sb, bq_sb, qT), (wk_sb, bk_sb, kT)):
                    for p in range(2):
                        ps = pxp.tile([128, TQ], F32, tag="x", name="qk_ps")
                        for k in range(NCK):
                            nc.tensor.matmul(
                                ps[:],
                                lhsT=wsb[k][:, p * 128 : (p + 1) * 128],
                                rhs=xt_sb[k][:, j * TQ : (j + 1) * TQ],
                                start=(k == 0),
                                stop=(k == NCK - 1),
                            )
                        nc.vector.tensor_scalar_add(
                            out=dst[p][:, j * TQ : (j + 1) * TQ],
                            in0=ps[:],
                            scalar1=bsb[p][:],
                        )
                # v tiles covered by this column chunk
                for t in range(4 * j, 4 * j + 4):
                    ps = pxp.tile([128, TQ], F32, tag="x", name="v_ps")
                    for k in range(NCK):
                        nc.tensor.matmul(
                            ps[:, 0 : G * DH],
                            lhsT=xt_sb[k][:, t * 128 : (t + 1) * 128],
                            rhs=wv_sb[k][:],
                            start=(k == 0),
                            stop=(k == NCK - 1),
                        )
                    va = vaug[t].rearrange("p (h x) -> p h x", x=65)
                    nc.vector.tensor_add(
                        out=va[:, :, 0:64],
                        in0=ps[:, 0 : G * DH].rearrange("p (h x) -> p h x", x=64),
                        in1=bvb_sb[:].rearrange("p (h x) -> p h x", x=64),
                    )
                    nc.sync.dma_start(
                        out=va[:, :, 64:65],
                        in_=onesb_d[:, 0:G].rearrange("p (h x) -> p h x", x=1),
                    )

            def emit_attention(jq):
                kmax = 4 * jq + 4
                recs = rp_map.setdefault(jq, {})
                for p in range(2):
                    ov = [
                        ovp.tile([65, TQ], F32, tag="ov", name="ovA"),
                        ovp.tile([65, TQ], F32, tag="ov", name="ovB"),
                    ]

                    def emit_v(kt, qlo, es):
                        va = vaug[kt].rearrange("p (h x) -> p h x", x=65)
                        for half in range(2):
                            nc.tensor.matmul(
                                ov[half][:, qlo:TQ],
                                lhsT=va[:, 2 * p + half, :],
                                rhs=es[:, half * TQ + qlo : half * TQ + TQ],
                                start=(kt == 0),
                                stop=(kt == kmax - 1),
                            )

                    prev = None
                    for kt in range(kmax):
                        # diagonal tiles only contribute to q >= k: narrow
                        # the S-matmul/exp/mask/V to the valid q-range
                        d = kt - 4 * jq
                        qlo = 128 * d if d >= 0 else 0
                        w = TQ - qlo
                        sps = sp0.tile([128, 2 * TQ], F32, tag="s", name="sps")
                        for half in range(2):
                            r = 64 * half
                            nc.tensor.matmul(
                                sps[:, half * TQ + qlo : half * TQ + TQ],
                                lhsT=kT[p][
                                    r : r + 64, kt * 128 : (kt + 1) * 128
                                ],
                                rhs=qT[p][
                                    r : r + 64,
                                    jq * TQ + qlo : (jq + 1) * TQ,
                                ],
                                start=True,
                                stop=True,
                            )
                        es = esp.tile([128, 2 * TQ], ATT_DT, tag="es", name="es")
                        sps_v = sps.rearrange("p (h q) -> p h q", q=TQ)
                        es_v = es.rearrange("p (h q) -> p h q", q=TQ)
                        nc.scalar.activation(
                            out=es_v[:, :, qlo:TQ],
                            in_=sps_v[:, :, qlo:TQ],
                            func=mybir.ActivationFunctionType.Exp,
                            scale=SCALE,
                        )
                        if d >= 0:
                            # es col qlo+t is q-offset t past the diagonal
                            # start: valid iff t >= k
                            mask_v = mask.rearrange("p (h q) -> p h q", q=TQ)
                            nc.vector.tensor_mul(
                                out=es_v[:, :, qlo:TQ],
                                in0=es_v[:, :, qlo:TQ],
                                in1=mask_v[:, :, 0 : TQ - qlo],
                            )
                        if prev is not None:
                            emit_v(*prev)
                        prev = (kt, qlo, es)
                    emit_v(*prev)
                    # epilogue: move unnormalized O out; reciprocal of the
                    # denominator row straight from PSUM (no DMA chain)
                    for half in range(2):
                        nc.vector.tensor_copy(
                            out=oT[p][
                                64 * half : 64 * half + 64,
                                jq * TQ : (jq + 1) * TQ,
                            ],
                            in_=ov[half][0:64, :],
                        )
                        rec_t = rpp.tile([1, TQ], F32R, tag="rp", name="rp")
                        nc.vector.reciprocal(out=rec_t[:], in_=ov[half][64:65, :])
                        recs[(p, half)] = rec_t

            def emit_taila(jq):
                # normalize this column block (broadcast 1/den over the 64
                # head rows via a K=1 matmul), ship to AllGather
                for p in range(2):
                    for half in range(2):
                        rec_t = rp_map[jq][(p, half)]
                        recb = pxp.tile([128, TQ], F32, tag="x", name="recb")
                        nc.tensor.matmul(
                            recb[0:64, :],
                            lhsT=ones_sb[0:1, :],
                            rhs=rec_t[:],
                            start=True,
                            stop=True,
                        )
                        dst = oT[p][
                            64 * half : 64 * half + 64, jq * TQ : (jq + 1) * TQ
                        ]
                        nc.vector.tensor_mul(out=dst, in0=dst, in1=recb[0:64, :])
                    nc.sync.dma_start(
                        out=ag_in[jq][p * 128 : (p + 1) * 128, :],
                        in_=oT[p][:, jq * TQ : (jq + 1) * TQ],
                    )
                nc.gpsimd.collective_compute(
                    "AllGather",
                    mybir.AluOpType.bypass,
                    ins=[ag_in[jq][:]],
                    outs=[ag_out[jq][:]],
                    replica_groups=GROUPS,
                )

            def emit_tailb(jq):
                # gathered O^T back to SBUF, then the full-contraction
                # projection for this core's 256 output channels
                og = ogp.tile([128, NCK * TQ], ATT_DT, tag="og", name="og")
                nc.gpsimd.dma_start(
                    out=og.rearrange("p (k q) -> p k q", q=TQ),
                    in_=ag_out[jq].rearrange("(k p) q -> p k q", p=128),
                )
                for rt in range(2):
                    ps = pxp.tile([128, TQ], F32, tag="x", name="pmm")
                    for k in range(NCK):
                        nc.tensor.matmul(
                            ps[:],
                            lhsT=wp_sb[k][:, rt * 128 : (rt + 1) * 128],
                            rhs=og[:, k * TQ : (k + 1) * TQ],
                            start=(k == 0),
                            stop=(k == NCK - 1),
                        )
                    o = ocp.tile([128, TQ], F32, tag="oc", name="oc")
                    nc.vector.tensor_scalar_add(
                        out=o[:], in0=ps[:], scalar1=bp_sb[rt][:]
                    )
                    nc.sync.dma_start(
                        out=y_d[rt * 128 : (rt + 1) * 128, jq * TQ : (jq + 1) * TQ],
                        in_=o[:],
                    )

            emit_qkv(0)
            for jq in range(NJQ):
                emit_attention(jq)
                emit_taila(jq)
                if jq + 1 < NJQ:
                    emit_qkv(jq + 1)
                if jq >= 1:
                    emit_tailb(jq - 1)
            emit_tailb(NJQ - 1)

    nc.compile()
    return nc


def _get_program():
    global _PROG
    if _PROG is None:
        _PROG = _build_program()
    return _PROG


def kernel(x, W_qkv, b_qkv, W_proj, b_proj):
    x = np.asarray(x, dtype=np.float32)
    W_qkv = np.asarray(W_qkv, dtype=np.float32)
    b_qkv = np.asarray(b_qkv, dtype=np.float32)
    W_proj = np.asarray(W_proj, dtype=np.float32)
    b_proj = np.asarray(b_proj, dtype=np.float32)

    nc = _get_program()

    mm_np = np.float16
    att_np = np.float16
    u = np.arange(TQ)[None, :]
    kl = np.arange(128)[:, None]
    m512 = (u >= kl).astype(att_np)
    mask_host = np.concatenate([m512, m512], axis=1)
    ones_host = np.ones((128, 64), dtype=np.float32)
    onesb_host = np.ones((128, 64), dtype=att_np)

    xts = [np.ascontiguousarray(x[b].T).astype(mm_np) for b in range(B)]
    in_maps = []
    for c in range(N_CORES):
        b, g = divmod(c, 4)
        cs = slice(g * G * DH, (g + 1) * G * DH)
        in_maps.append(
            {
                "xt": xts[b],
                "wq": np.ascontiguousarray(W_qkv[:, cs]).astype(mm_np),
                "wk": np.ascontiguousarray(W_qkv[:, C:][:, cs]).astype(mm_np),
                "wv": np.ascontiguousarray(W_qkv[:, 2 * C :][:, cs]).astype(mm_np),
                "wp": np.ascontiguousarray(W_proj[:, cs]).astype(mm_np),
                "bq": np.ascontiguousarray(b_qkv[cs]).reshape(-1, 1),
                "bk": np.ascontiguousarray(b_qkv[C:][cs]).reshape(-1, 1),
                "bv": np.ascontiguousarray(b_qkv[2 * C :][cs]).reshape(1, -1),
                "bp": np.ascontiguousarray(
                    b_proj[cs]
                ).reshape(-1, 1),
                "mask": mask_host,
                "ones": ones_host,
                "onesb": onesb_host,
            }
        )

    global _last_in_maps
    _last_in_maps = in_maps
    res = run_bass_kernel_spmd(nc, in_maps, list(range(N_CORES)))

    y = np.empty((B, T, C), dtype=np.float32)
    for b in range(B):
        yT = np.concatenate(
            [res.results[4 * b + r]["y"] for r in range(4)], axis=0
        )
        y[b] = yT.T
    return y


# revision 16
# speedup vs baseline: 1.0571x; 1.0571x over previous
"""Causal self-attention on 8 Trainium2 NeuronCores.

Reference (fp32):
    qkv = x @ W_qkv + b_qkv ; split q,k,v ; heads H=16, Dh=64
    scores = q @ k^T / sqrt(Dh), causal mask, softmax
    out = (attn @ v) re-merged ; y = out @ W_proj + b_proj

Sharding: tensor-parallel over heads x data-parallel over batch.
Core c (0..7) owns batch b = c//4 and head group g = c%4 (heads 4g..4g+3).
Each core computes q^T,k^T,v for its 4 heads from x[b]^T, runs causal
attention (scores transposed layout, exp without max-subtraction -- scores
are O(5) so fp32 exp is safe, denominator via an appended ones-column in
the V matmul).

v2 changes vs the ReduceScatter baseline:
  - per-512-column pipeline: qkv chunk j is emitted together with
    attention chunk j so the tensor engine never drains between phases.
  - combine across the 4 cores of a batch via fp16 AllGather of the
    normalized O^T (256 KiB in / 1 MiB out per chunk) instead of fp32
    ReduceScatter of partial y (2 MiB per chunk); each core then runs the
    full-contraction projection for its 256 output channels.
  - exp batched: both heads of a pair share one [128, 1024] PSUM score
    tile and a single Exp activation (halves ACT instruction overhead).
  - ScalarE carries only the exp stream; all DMA issues live on the
    sync/gpsimd queues.

Matmuls run in fp16 (full PE speed + fast weight load, ~3e-4 rel err
end to end vs the fp32 reference).
"""

import numpy as np

import concourse.bacc as bacc
import concourse.mybir as mybir
import concourse.tile as tile
from concourse.bass_utils import run_bass_kernel_spmd

B = 2
T = 2048
C = 1024
H = 16
DH = 64
G = 4  # heads per core
N_CORES = 8
TQ = 512  # q-chunk width
NKT = T // 128  # k tiles per head
NJQ = T // TQ  # q chunks
NCK = C // 128  # contraction tiles over model dim
SCALE = 1.0 / np.sqrt(DH)
GROUPS = [[0, 1, 2, 3], [4, 5, 6, 7]]

F32 = mybir.dt.float32
F32R = mybir.dt.float32r
BF16 = mybir.dt.bfloat16
FP16 = mybir.dt.float16
ATT_DT = FP16
MM_DT = FP16

_PROG = None


def _build_program():
    nc = bacc.Bacc(
        "TRN2", target_bir_lowering=False, debug=False, num_devices=N_CORES
    )
    xt_d = nc.dram_tensor("xt", [C, T], MM_DT, kind="ExternalInput").ap()
    wq_d = nc.dram_tensor("wq", [C, G * DH], MM_DT, kind="ExternalInput").ap()
    wk_d = nc.dram_tensor("wk", [C, G * DH], MM_DT, kind="ExternalInput").ap()
    wv_d = nc.dram_tensor("wv", [C, G * DH], MM_DT, kind="ExternalInput").ap()
    # W_proj[:, 256g:256g+256] -- full contraction dim, this core's out cols
    wp_d = nc.dram_tensor("wp", [C, C // 4], MM_DT, kind="ExternalInput").ap()
    bq_d = nc.dram_tensor("bq", [G * DH, 1], F32, kind="ExternalInput").ap()
    bk_d = nc.dram_tensor("bk", [G * DH, 1], F32, kind="ExternalInput").ap()
    bv_d = nc.dram_tensor("bv", [1, G * DH], F32, kind="ExternalInput").ap()
    bp_d = nc.dram_tensor("bp", [C // 4, 1], F32, kind="ExternalInput").ap()
    # maskw[k, h*512 + u] = (u >= k), h = 0,1 -- one mul covers both heads
    mask_d = nc.dram_tensor("mask", [128, 1024], ATT_DT, kind="ExternalInput").ap()
    bc2_d = nc.dram_tensor("bc2", [2, 128], F32R, kind="ExternalInput").ap()
    ones_d = nc.dram_tensor("ones", [128, 64], F32R, kind="ExternalInput").ap()
    onesb_d = nc.dram_tensor("onesb", [128, 64], ATT_DT, kind="ExternalInput").ap()
    ag_in = [
        nc.dram_tensor(f"ag_in{j}", [C // 4, TQ], ATT_DT).ap() for j in range(NJQ)
    ]
    ag_out = [
        nc.dram_tensor(f"ag_out{j}", [C, TQ], ATT_DT).ap() for j in range(NJQ)
    ]
    y_d = nc.dram_tensor("y", [C // 4, T], F32, kind="ExternalOutput").ap()

    with tile.TileContext(nc) as tc:
        with (
            nc.allow_low_precision(reason="fp16 matmul pipeline by design"),
            tc.tile_pool(name="ll", bufs=1) as ll,
            tc.tile_pool(name="rp", bufs=4) as rpp,
            tc.tile_pool(name="es", bufs=4) as esp,
            tc.tile_pool(name="oc", bufs=4) as ocp,
            tc.tile_pool(name="og", bufs=2) as ogp,
            tc.tile_pool(name="ps0", bufs=2, space="PSUM") as sp0,
            tc.tile_pool(name="ov", bufs=2, space="PSUM") as ovp,
            tc.tile_pool(name="px", bufs=2, space="PSUM") as pxp,
        ):
            # ---- long-lived tiles -------------------------------------
            qT = [ll.tile([128, T], ATT_DT, tag=f"qT{p}", name=f"qT{p}") for p in range(2)]
            kT = [ll.tile([128, T], ATT_DT, tag=f"kT{p}", name=f"kT{p}") for p in range(2)]
            oT = [ll.tile([128, T], ATT_DT, tag=f"oT{p}", name=f"oT{p}") for p in range(2)]
            vaug = [ll.tile([128, G * 65], ATT_DT, tag=f"va{t}", name=f"va{t}") for t in range(NKT)]

            mask = ll.tile([128, 1024], ATT_DT, tag="mask")
            nc.sync.dma_start(out=mask[:], in_=mask_d[:])
            bc2_sb = ll.tile([2, 128], F32R, tag="bc2")
            nc.sync.dma_start(out=bc2_sb[:], in_=bc2_d[:])
            ones_sb = ll.tile([128, 64], F32R, tag="ones")
            nc.sync.dma_start(out=ones_sb[:], in_=ones_d[:])
            bq_sb = [ll.tile([128, 1], F32, tag=f"bq{p}", name=f"bq{p}") for p in range(2)]
            bk_sb = [ll.tile([128, 1], F32, tag=f"bk{p}", name=f"bk{p}") for p in range(2)]
            for p in range(2):
                nc.sync.dma_start(
                    out=bq_sb[p][:], in_=bq_d[p * 128 : (p + 1) * 128, :]
                )
                nc.sync.dma_start(
                    out=bk_sb[p][:], in_=bk_d[p * 128 : (p + 1) * 128, :]
                )
            bv_sb = ll.tile([1, G * DH], F32, tag="bv")
            nc.sync.dma_start(out=bv_sb[:], in_=bv_d[:])
            bp_sb = [ll.tile([128, 1], F32, tag=f"bp{i}", name=f"bp{i}") for i in range(2)]
            for i in range(2):
                nc.sync.dma_start(
                    out=bp_sb[i][:], in_=bp_d[i * 128 : (i + 1) * 128, :]
                )
            # projection weights: 8 x [128, 256], loaded late (first use ~40us in)
            wp_sb = [
                ll.tile([128, C // 4], MM_DT, tag=f"wp{k}", name=f"wp{k}")
                for k in range(NCK)
            ]
            for k in range(NCK):
                nc.gpsimd.dma_start(
                    out=wp_sb[k][:], in_=wp_d[k * 128 : (k + 1) * 128, :]
                )

            # bv broadcast across partitions (via ones-row matmul)
            ones_row = ll.tile([1, 128], F32R, tag="ones_row")
            nc.sync.dma_start(out=ones_row[:, 0:64], in_=ones_d[0:1, :])
            nc.sync.dma_start(out=ones_row[:, 64:128], in_=ones_d[0:1, :])
            bv_r = ll.tile([1, G * DH], F32R, tag="bvr")
            nc.vector.tensor_copy(out=bv_r[:], in_=bv_sb[:])
            bvb_ps = pxp.tile([128, TQ], F32, tag="x", name="bvb_ps")
            bvb_sb = ll.tile([128, G * DH], F32, tag="bvb")
            nc.tensor.matmul(
                bvb_ps[:, 0 : G * DH], lhsT=ones_row[:], rhs=bv_r[:], start=True, stop=True
            )
            nc.vector.tensor_copy(out=bvb_sb[:], in_=bvb_ps[:, 0 : G * DH])

            # ---- input / weight streaming -----------------------------
            wq_sb, wk_sb, wv_sb = [], [], []
            xt_sb = [
                ll.tile([128, T], MM_DT, tag=f"xt{k}", name=f"xt{k}")
                for k in range(NCK)
            ]
            # j=0 column of xt + the qkv weights, interleaved k-major on two
            # queues, so the first qkv chains unblock as early as possible
            for k in range(NCK):
                nc.sync.dma_start(
                    out=xt_sb[k][:, 0:TQ],
                    in_=xt_d[k * 128 : (k + 1) * 128, 0:TQ],
                )
                for qi, (name, dst, src) in enumerate(
                    (
                        ("q", wq_sb, wq_d),
                        ("k", wk_sb, wk_d),
                        ("v", wv_sb, wv_d),
                    )
                ):
                    t = ll.tile([128, G * DH], MM_DT, tag=f"w{name}{k}", name=f"w{name}{k}")
                    nc.gpsimd.dma_start(
                        out=t[:], in_=src[k * 128 : (k + 1) * 128, :]
                    )
                    dst.append(t)
            for j in range(1, NJQ):
                for k in range(NCK):
                    eng = (nc.sync, nc.gpsimd)[k % 2]
                    eng.dma_start(
                        out=xt_sb[k][:, j * TQ : (j + 1) * TQ],
                        in_=xt_d[k * 128 : (k + 1) * 128, j * TQ : (j + 1) * TQ],
                    )

            rp_map = {}

            def emit_qkv(j):
                # q^T / k^T chains for this column chunk
                for wsb, bsb, dst in ((wq_sb, bq_sb, qT), (wk_sb, bk_sb, kT)):
                    for p in range(2):
                        ps = pxp.tile([128, TQ], F32, tag="x", name="qk_ps")
                        for k in range(NCK):
                            nc.tensor.matmul(
                                ps[:],
                                lhsT=wsb[k][:, p * 128 : (p + 1) * 128],
                                rhs=xt_sb[k][:, j * TQ : (j + 1) * TQ],
                                start=(k == 0),
                                stop=(k == NCK - 1),
                            )
                        nc.vector.tensor_scalar_add(
                            out=dst[p][:, j * TQ : (j + 1) * TQ],
                            in0=ps[:],
                            scalar1=bsb[p][:],
                        )
                # v tiles covered by this column chunk
                for t in range(4 * j, 4 * j + 4):
                    ps = pxp.tile([128, TQ], F32, tag="x", name="v_ps")
                    for k in range(NCK):
                        nc.tensor.matmul(
                            ps[:, 0 : G * DH],
                            lhsT=xt_sb[k][:, t * 128 : (t + 1) * 128],
                            rhs=wv_sb[k][:],
                            start=(k == 0),
                            stop=(k == NCK - 1),
                        )
                    va = vaug[t].rearrange("p (h x) -> p h x", x=65)
                    nc.vector.tensor_add(
                        out=va[:, :, 0:64],
                        in0=ps[:, 0 : G * DH].rearrange("p (h x) -> p h x", x=64),
                        in1=bvb_sb[:].rearrange("p (h x) -> p h x", x=64),
                    )
                    nc.sync.dma_start(
                        out=va[:, :, 64:65],
                        in_=onesb_d[:, 0:G].rearrange("p (h x) -> p h x", x=1),
                    )

            def emit_attention(jq):
                kmax = 4 * jq + 4
                den4 = rpp.tile([4, TQ], F32, tag="den4", name="den4")
                for p in range(2):
                    ov = [
                        ovp.tile([65, TQ], F32, tag="ov", name="ovA"),
                        ovp.tile([65, TQ], F32, tag="ov", name="ovB"),
                    ]

                    def emit_v(kt, qlo, es):
                        va = vaug[kt].rearrange("p (h x) -> p h x", x=65)
                        for half in range(2):
                            nc.tensor.matmul(
                                ov[half][:, qlo:TQ],
                                lhsT=va[:, 2 * p + half, :],
                                rhs=es[:, half * TQ + qlo : half * TQ + TQ],
                                start=(kt == 0),
                                stop=(kt == kmax - 1),
                            )

                    prev = None
                    for kt in range(kmax):
                        # diagonal tiles only contribute to q >= k: narrow
                        # the S-matmul/exp/mask/V to the valid q-range
                        d = kt - 4 * jq
                        qlo = 128 * d if d >= 0 else 0
                        w = TQ - qlo
                        sps = sp0.tile([128, 2 * TQ], F32, tag="s", name="sps")
                        for half in range(2):
                            r = 64 * half
                            nc.tensor.matmul(
                                sps[:, half * TQ + qlo : half * TQ + TQ],
                                lhsT=kT[p][
                                    r : r + 64, kt * 128 : (kt + 1) * 128
                                ],
                                rhs=qT[p][
                                    r : r + 64,
                                    jq * TQ + qlo : (jq + 1) * TQ,
                                ],
                                start=True,
                                stop=True,
                            )
                        es = esp.tile([128, 2 * TQ], ATT_DT, tag="es", name="es")
                        sps_v = sps.rearrange("p (h q) -> p h q", q=TQ)
                        es_v = es.rearrange("p (h q) -> p h q", q=TQ)
                        nc.scalar.activation(
                            out=es_v[:, :, qlo:TQ],
                            in_=sps_v[:, :, qlo:TQ],
                            func=mybir.ActivationFunctionType.Exp,
                            scale=SCALE,
                        )
                        if d >= 0:
                            # es col qlo+t is q-offset t past the diagonal
                            # start: valid iff t >= k
                            mask_v = mask.rearrange("p (h q) -> p h q", q=TQ)
                            nc.vector.tensor_mul(
                                out=es_v[:, :, qlo:TQ],
                                in0=es_v[:, :, qlo:TQ],
                                in1=mask_v[:, :, 0 : TQ - qlo],
                            )
                        if prev is not None:
                            emit_v(*prev)
                        prev = (kt, qlo, es)
                    emit_v(*prev)
                    # epilogue: move unnormalized O and denominators out
                    for half in range(2):
                        nc.vector.tensor_copy(
                            out=oT[p][
                                64 * half : 64 * half + 64,
                                jq * TQ : (jq + 1) * TQ,
                            ],
                            in_=ov[half][0:64, :],
                        )
                        dt_t = rpp.tile([1, TQ], F32, tag="dt", name="dt")
                        nc.vector.tensor_copy(
                            out=dt_t[:], in_=ov[half][64:65, :]
                        )
                        nc.sync.dma_start(
                            out=den4[2 * p + half : 2 * p + half + 1, :],
                            in_=dt_t[:],
                        )
                rec4 = rpp.tile([4, TQ], F32R, tag="rec4", name="rec4")
                nc.vector.reciprocal(out=rec4[:], in_=den4[:])
                rp_ts = []
                for p in range(2):
                    rp_t = rpp.tile([2, TQ], F32R, tag="rp", name="rp")
                    nc.sync.dma_start(
                        out=rp_t[:], in_=rec4[2 * p : 2 * p + 2, :]
                    )
                    rp_ts.append(rp_t)
                rp_map[jq] = rp_ts

            def emit_taila(jq):
                # normalize this column block (broadcast 1/den over the 64
                # head rows via a K=1 matmul), ship to AllGather
                for p in range(2):
                    rp_t = rp_map[jq][p]
                    recb = pxp.tile([128, TQ], F32, tag="x", name="recb")
                    nc.tensor.matmul(
                        recb[:],
                        lhsT=bc2_sb[:],
                        rhs=rp_t[:],
                        start=True,
                        stop=True,
                    )
                    dst = oT[p][:, jq * TQ : (jq + 1) * TQ]
                    nc.vector.tensor_mul(out=dst, in0=dst, in1=recb[:])
                    nc.sync.dma_start(
                        out=ag_in[jq][p * 128 : (p + 1) * 128, :], in_=dst
                    )
                nc.gpsimd.collective_compute(
                    "AllGather",
                    mybir.AluOpType.bypass,
                    ins=[ag_in[jq][:]],
                    outs=[ag_out[jq][:]],
                    replica_groups=GROUPS,
                )

            def emit_tailb(jq):
                # gathered O^T back to SBUF, then the full-contraction
                # projection for this core's 256 output channels
                og = ogp.tile([128, NCK * TQ], ATT_DT, tag="og", name="og")
                nc.gpsimd.dma_start(
                    out=og.rearrange("p (k q) -> p k q", q=TQ),
                    in_=ag_out[jq].rearrange("(k p) q -> p k q", p=128),
                )
                for rt in range(2):
                    ps = pxp.tile([128, TQ], F32, tag="x", name="pmm")
                    for k in range(NCK):
                        nc.tensor.matmul(
                            ps[:],
                            lhsT=wp_sb[k][:, rt * 128 : (rt + 1) * 128],
                            rhs=og[:, k * TQ : (k + 1) * TQ],
                            start=(k == 0),
                            stop=(k == NCK - 1),
                        )
                    o = ocp.tile([128, TQ], F32, tag="oc", name="oc")
                    nc.vector.tensor_scalar_add(
                        out=o[:], in0=ps[:], scalar1=bp_sb[rt][:]
                    )
                    nc.sync.dma_start(
                        out=y_d[rt * 128 : (rt + 1) * 128, jq * TQ : (jq + 1) * TQ],
                        in_=o[:],
                    )

            for jq in range(NJQ):
                emit_qkv(jq)
                emit_attention(jq)
                emit_taila(jq)
                if jq >= 2:
                    emit_tailb(jq - 2)
            emit_tailb(NJQ - 2)
            emit_tailb(NJQ - 1)

    nc.compile()
    return nc


def _get_program():
    global _PROG
    if _PROG is None:
        _PROG = _build_program()
    return _PROG


def kernel(x, W_qkv, b_qkv, W_proj, b_proj):
    x = np.asarray(x, dtype=np.float32)
    W_qkv = np.asarray(W_qkv, dtype=np.float32)
    b_qkv = np.asarray(b_qkv, dtype=np.float32)
    W_proj = np.asarray(W_proj, dtype=np.float32)
    b_proj = np.asarray(b_proj, dtype=np.float32)

    nc = _get_program()

    mm_np = np.float16
    att_np = np.float16
    u = np.arange(TQ)[None, :]
    kl = np.arange(128)[:, None]
    m512 = (u >= kl).astype(att_np)
    mask_host = np.concatenate([m512, m512], axis=1)
    ones_host = np.ones((128, 64), dtype=np.float32)
    onesb_host = np.ones((128, 64), dtype=att_np)

    bc2_host = np.zeros((2, 128), dtype=np.float32)
    bc2_host[0, 0:64] = 1.0
    bc2_host[1, 64:128] = 1.0

    xts = [np.ascontiguousarray(x[b].T).astype(mm_np) for b in range(B)]
    in_maps = []
    for c in range(N_CORES):
        b, g = divmod(c, 4)
        cs = slice(g * G * DH, (g + 1) * G * DH)
        in_maps.append(
            {
                "xt": xts[b],
                "wq": np.ascontiguousarray(W_qkv[:, cs]).astype(mm_np),
                "wk": np.ascontiguousarray(W_qkv[:, C:][:, cs]).astype(mm_np),
                "wv": np.ascontiguousarray(W_qkv[:, 2 * C :][:, cs]).astype(mm_np),
                "wp": np.ascontiguousarray(W_proj[:, cs]).astype(mm_np),
                "bq": np.ascontiguousarray(b_qkv[cs]).reshape(-1, 1),
                "bk": np.ascontiguousarray(b_qkv[C:][cs]).reshape(-1, 1),
                "bv": np.ascontiguousarray(b_qkv[2 * C :][cs]).reshape(1, -1),
                "bp": np.ascontiguousarray(
                    b_proj[cs]
                ).reshape(-1, 1),
                "mask": mask_host,
                "bc2": bc2_host,
                "ones": ones_host,
                "onesb": onesb_host,
            }
        )

    global _last_in_maps
    _last_in_maps = in_maps
    res = run_bass_kernel_spmd(nc, in_maps, list(range(N_CORES)))

    y = np.empty((B, T, C), dtype=np.float32)
    for b in range(B):
        yT = np.concatenate(
            [res.results[4 * b + r]["y"] for r in range(4)], axis=0
        )
        y[b] = yT.T
    return y


# revision 18
# speedup vs baseline: 1.0881x; 1.0293x over previous
"""Causal self-attention on 8 Trainium2 NeuronCores.

Reference (fp32):
    qkv = x @ W_qkv + b_qkv ; split q,k,v ; heads H=16, Dh=64
    scores = q @ k^T / sqrt(Dh), causal mask, softmax
    out = (attn @ v) re-merged ; y = out @ W_proj + b_proj

Sharding: tensor-parallel over heads x data-parallel over batch.
Core c (0..7) owns batch b = c//4 and head group g = c%4 (heads 4g..4g+3).
Each core computes q^T,k^T,v for its 4 heads from x[b]^T, runs causal
attention (scores transposed layout, exp without max-subtraction -- scores
are O(5) so fp32 exp is safe, denominator via an appended ones-column in
the V matmul).

v2 changes vs the ReduceScatter baseline:
  - per-512-column pipeline: qkv chunk j is emitted together with
    attention chunk j so the tensor engine never drains between phases.
  - combine across the 4 cores of a batch via fp16 AllGather of the
    normalized O^T (256 KiB in / 1 MiB out per chunk) instead of fp32
    ReduceScatter of partial y (2 MiB per chunk); each core then runs the
    full-contraction projection for its 256 output channels.
  - exp batched: both heads of a pair share one [128, 1024] PSUM score
    tile and a single Exp activation (halves ACT instruction overhead).
  - ScalarE carries only the exp stream; all DMA issues live on the
    sync/gpsimd queues.

Matmuls run in fp16 (full PE speed + fast weight load, ~3e-4 rel err
end to end vs the fp32 reference).
"""

import numpy as np

import concourse.bacc as bacc
import concourse.mybir as mybir
import concourse.tile as tile
from concourse.bass_utils import run_bass_kernel_spmd

B = 2
T = 2048
C = 1024
H = 16
DH = 64
G = 4  # heads per core
N_CORES = 8
TQ = 512  # q-chunk width
NKT = T // 128  # k tiles per head
NJQ = T // TQ  # q chunks
NCK = C // 128  # contraction tiles over model dim
SCALE = 1.0 / np.sqrt(DH)
GROUPS = [[0, 1, 2, 3], [4, 5, 6, 7]]

F32 = mybir.dt.float32
F32R = mybir.dt.float32r
BF16 = mybir.dt.bfloat16
FP16 = mybir.dt.float16
ATT_DT = FP16
MM_DT = FP16

_PROG = None


def _build_program():
    nc = bacc.Bacc(
        "TRN2", target_bir_lowering=False, debug=False, num_devices=N_CORES
    )
    xt_d = nc.dram_tensor("xt", [C, T], MM_DT, kind="ExternalInput").ap()
    wq_d = nc.dram_tensor("wq", [C, G * DH], MM_DT, kind="ExternalInput").ap()
    wk_d = nc.dram_tensor("wk", [C, G * DH], MM_DT, kind="ExternalInput").ap()
    wv_d = nc.dram_tensor("wv", [C, G * DH], MM_DT, kind="ExternalInput").ap()
    # W_proj[:, 256g:256g+256] -- full contraction dim, this core's out cols
    wp_d = nc.dram_tensor("wp", [C, C // 4], MM_DT, kind="ExternalInput").ap()
    bq_d = nc.dram_tensor("bq", [G * DH, 1], F32, kind="ExternalInput").ap()
    bk_d = nc.dram_tensor("bk", [G * DH, 1], F32, kind="ExternalInput").ap()
    bv_d = nc.dram_tensor("bv", [1, G * DH], F32, kind="ExternalInput").ap()
    bp_d = nc.dram_tensor("bp", [C // 4, 1], F32, kind="ExternalInput").ap()
    # maskw[k, h*512 + u] = (u >= k), h = 0,1 -- one mul covers both heads
    mask_d = nc.dram_tensor("mask", [128, 1024], ATT_DT, kind="ExternalInput").ap()
    bc2_d = nc.dram_tensor("bc2", [2, 128], F32R, kind="ExternalInput").ap()
    ones_d = nc.dram_tensor("ones", [128, 64], F32R, kind="ExternalInput").ap()
    onesb_d = nc.dram_tensor("onesb", [128, 64], ATT_DT, kind="ExternalInput").ap()
    ag_in = [
        nc.dram_tensor(f"ag_in{j}", [C // 4, TQ], ATT_DT).ap() for j in range(NJQ)
    ]
    ag_out = [
        nc.dram_tensor(f"ag_out{j}", [C, TQ], ATT_DT).ap() for j in range(NJQ)
    ]
    y_d = nc.dram_tensor("y", [C // 4, T], F32, kind="ExternalOutput").ap()

    with tile.TileContext(nc) as tc:
        with (
            nc.allow_low_precision(reason="fp16 matmul pipeline by design"),
            tc.tile_pool(name="ll", bufs=1) as ll,
            tc.tile_pool(name="rp", bufs=4) as rpp,
            tc.tile_pool(name="es", bufs=4) as esp,
            tc.tile_pool(name="oc", bufs=4) as ocp,
            tc.tile_pool(name="og", bufs=2) as ogp,
            tc.tile_pool(name="ps0", bufs=2, space="PSUM") as sp0,
            tc.tile_pool(name="ov", bufs=2, space="PSUM") as ovp,
            tc.tile_pool(name="px", bufs=2, space="PSUM") as pxp,
        ):
            # ---- long-lived tiles -------------------------------------
            qT = [ll.tile([128, T], ATT_DT, tag=f"qT{p}", name=f"qT{p}") for p in range(2)]
            kT = [ll.tile([128, T], ATT_DT, tag=f"kT{p}", name=f"kT{p}") for p in range(2)]
            oT = [ll.tile([128, T], ATT_DT, tag=f"oT{p}", name=f"oT{p}") for p in range(2)]
            vaug = [ll.tile([128, G * 65], ATT_DT, tag=f"va{t}", name=f"va{t}") for t in range(NKT)]

            mask = ll.tile([128, 1024], ATT_DT, tag="mask")
            nc.sync.dma_start(out=mask[:], in_=mask_d[:])
            bc2_sb = ll.tile([2, 128], F32R, tag="bc2")
            nc.sync.dma_start(out=bc2_sb[:], in_=bc2_d[:])
            ones_sb = ll.tile([128, 64], F32R, tag="ones")
            nc.sync.dma_start(out=ones_sb[:], in_=ones_d[:])
            bq_sb = [ll.tile([128, 1], F32, tag=f"bq{p}", name=f"bq{p}") for p in range(2)]
            bk_sb = [ll.tile([128, 1], F32, tag=f"bk{p}", name=f"bk{p}") for p in range(2)]
            for p in range(2):
                nc.sync.dma_start(
                    out=bq_sb[p][:], in_=bq_d[p * 128 : (p + 1) * 128, :]
                )
                nc.sync.dma_start(
                    out=bk_sb[p][:], in_=bk_d[p * 128 : (p + 1) * 128, :]
                )
            bv_sb = ll.tile([1, G * DH], F32, tag="bv")
            nc.sync.dma_start(out=bv_sb[:], in_=bv_d[:])
            bp_sb = [ll.tile([128, 1], F32, tag=f"bp{i}", name=f"bp{i}") for i in range(2)]
            for i in range(2):
                nc.sync.dma_start(
                    out=bp_sb[i][:], in_=bp_d[i * 128 : (i + 1) * 128, :]
                )
            # projection weights: 8 x [128, 256], loaded late (first use ~40us in)
            wp_sb = [
                ll.tile([128, C // 4], MM_DT, tag=f"wp{k}", name=f"wp{k}")
                for k in range(NCK)
            ]
            for k in range(NCK):
                nc.gpsimd.dma_start(
                    out=wp_sb[k][:], in_=wp_d[k * 128 : (k + 1) * 128, :]
                )

            # bv broadcast across partitions (via ones-row matmul)
            ones_row = ll.tile([1, 128], F32R, tag="ones_row")
            nc.sync.dma_start(out=ones_row[:, 0:64], in_=ones_d[0:1, :])
            nc.sync.dma_start(out=ones_row[:, 64:128], in_=ones_d[0:1, :])
            bv_r = ll.tile([1, G * DH], F32R, tag="bvr")
            nc.vector.tensor_copy(out=bv_r[:], in_=bv_sb[:])
            bvb_ps = pxp.tile([128, TQ], F32, tag="x", name="bvb_ps")
            bvb_sb = ll.tile([128, G * DH], F32, tag="bvb")
            nc.tensor.matmul(
                bvb_ps[:, 0 : G * DH], lhsT=ones_row[:], rhs=bv_r[:], start=True, stop=True
            )
            nc.vector.tensor_copy(out=bvb_sb[:], in_=bvb_ps[:, 0 : G * DH])

            # ---- input / weight streaming -----------------------------
            wq_sb, wk_sb, wv_sb = [], [], []
            xt_sb = [
                ll.tile([128, T], MM_DT, tag=f"xt{k}", name=f"xt{k}")
                for k in range(NCK)
            ]
            # j=0 column of xt + the qkv weights, interleaved k-major on two
            # queues, so the first qkv chains unblock as early as possible
            for k in range(NCK):
                nc.sync.dma_start(
                    out=xt_sb[k][:, 0:TQ],
                    in_=xt_d[k * 128 : (k + 1) * 128, 0:TQ],
                )
                for qi, (name, dst, src) in enumerate(
                    (
                        ("q", wq_sb, wq_d),
                        ("k", wk_sb, wk_d),
                        ("v", wv_sb, wv_d),
                    )
                ):
                    t = ll.tile([128, G * DH], MM_DT, tag=f"w{name}{k}", name=f"w{name}{k}")
                    nc.gpsimd.dma_start(
                        out=t[:], in_=src[k * 128 : (k + 1) * 128, :]
                    )
                    dst.append(t)
            for j in range(1, NJQ):
                for k in range(NCK):
                    eng = (nc.sync, nc.gpsimd)[k % 2]
                    eng.dma_start(
                        out=xt_sb[k][:, j * TQ : (j + 1) * TQ],
                        in_=xt_d[k * 128 : (k + 1) * 128, j * TQ : (j + 1) * TQ],
                    )

            rp_map = {}

            def emit_qkv(j):
                # q^T / k^T chains for this column chunk
                for wsb, bsb, dst in ((wq_sb, bq_sb, qT), (wk_sb, bk_sb, kT)):
                    for p in range(2):
                        ps = pxp.tile([128, TQ], F32, tag="x", name="qk_ps")
                        for k in range(NCK):
                            nc.tensor.matmul(
                                ps[:],
                                lhsT=wsb[k][:, p * 128 : (p + 1) * 128],
                                rhs=xt_sb[k][:, j * TQ : (j + 1) * TQ],
                                start=(k == 0),
                                stop=(k == NCK - 1),
                            )
                        nc.vector.tensor_scalar_add(
                            out=dst[p][:, j * TQ : (j + 1) * TQ],
                            in0=ps[:],
                            scalar1=bsb[p][:],
                        )
                # v tiles covered by this column chunk
                for t in range(4 * j, 4 * j + 4):
                    ps = pxp.tile([128, TQ], F32, tag="x", name="v_ps")
                    for k in range(NCK):
                        nc.tensor.matmul(
                            ps[:, 0 : G * DH],
                            lhsT=xt_sb[k][:, t * 128 : (t + 1) * 128],
                            rhs=wv_sb[k][:],
                            start=(k == 0),
                            stop=(k == NCK - 1),
                        )
                    va = vaug[t].rearrange("p (h x) -> p h x", x=65)
                    nc.vector.tensor_add(
                        out=va[:, :, 0:64],
                        in0=ps[:, 0 : G * DH].rearrange("p (h x) -> p h x", x=64),
                        in1=bvb_sb[:].rearrange("p (h x) -> p h x", x=64),
                    )
                    nc.sync.dma_start(
                        out=va[:, :, 64:65],
                        in_=onesb_d[:, 0:G].rearrange("p (h x) -> p h x", x=1),
                    )

            def emit_attention(jq):
                kmax = 4 * jq + 4
                den4 = rpp.tile([4, TQ], F32, tag="den4", name="den4")
                for p in range(2):
                    ov = [
                        ovp.tile([65, TQ], F32, tag="ov", name="ovA"),
                        ovp.tile([65, TQ], F32, tag="ov", name="ovB"),
                    ]

                    def emit_v(kt, qlo, es):
                        va = vaug[kt].rearrange("p (h x) -> p h x", x=65)
                        for half in range(2):
                            nc.tensor.matmul(
                                ov[half][:, qlo:TQ],
                                lhsT=va[:, 2 * p + half, :],
                                rhs=es[:, half * TQ + qlo : half * TQ + TQ],
                                start=(kt == 0),
                                stop=(kt == kmax - 1),
                            )

                    prev = None
                    for kt in range(kmax):
                        # diagonal tiles only contribute to q >= k: narrow
                        # the S-matmul/exp/mask/V to the valid q-range
                        d = kt - 4 * jq
                        qlo = 128 * d if d >= 0 else 0
                        w = TQ - qlo
                        sps = sp0.tile([128, 2 * TQ], F32, tag="s", name="sps")
                        for half in range(2):
                            r = 64 * half
                            nc.tensor.matmul(
                                sps[:, half * TQ + qlo : half * TQ + TQ],
                                lhsT=kT[p][
                                    r : r + 64, kt * 128 : (kt + 1) * 128
                                ],
                                rhs=qT[p][
                                    r : r + 64,
                                    jq * TQ + qlo : (jq + 1) * TQ,
                                ],
                                start=True,
                                stop=True,
                            )
                        es = esp.tile([128, 2 * TQ], ATT_DT, tag="es", name="es")
                        sps_v = sps.rearrange("p (h q) -> p h q", q=TQ)
                        es_v = es.rearrange("p (h q) -> p h q", q=TQ)
                        nc.scalar.activation(
                            out=es_v[:, :, qlo:TQ],
                            in_=sps_v[:, :, qlo:TQ],
                            func=mybir.ActivationFunctionType.Exp,
                            scale=SCALE,
                        )
                        if d >= 0:
                            # es col qlo+t is q-offset t past the diagonal
                            # start: valid iff t >= k
                            mask_v = mask.rearrange("p (h q) -> p h q", q=TQ)
                            nc.vector.tensor_mul(
                                out=es_v[:, :, qlo:TQ],
                                in0=es_v[:, :, qlo:TQ],
                                in1=mask_v[:, :, 0 : TQ - qlo],
                            )
                        if prev is not None:
                            emit_v(*prev)
                        prev = (kt, qlo, es)
                    emit_v(*prev)
                    # epilogue: move unnormalized O and denominators out
                    for half in range(2):
                        nc.vector.tensor_copy(
                            out=oT[p][
                                64 * half : 64 * half + 64,
                                jq * TQ : (jq + 1) * TQ,
                            ],
                            in_=ov[half][0:64, :],
                        )
                        dt_t = rpp.tile([1, TQ], F32, tag="dt", name="dt")
                        nc.vector.tensor_copy(
                            out=dt_t[:], in_=ov[half][64:65, :]
                        )
                        nc.sync.dma_start(
                            out=den4[2 * p + half : 2 * p + half + 1, :],
                            in_=dt_t[:],
                        )
                rec4 = rpp.tile([4, TQ], F32R, tag="rec4", name="rec4")
                nc.vector.reciprocal(out=rec4[:], in_=den4[:])
                rp_ts = []
                for p in range(2):
                    rp_t = rpp.tile([2, TQ], F32R, tag="rp", name="rp")
                    nc.sync.dma_start(
                        out=rp_t[:], in_=rec4[2 * p : 2 * p + 2, :]
                    )
                    rp_ts.append(rp_t)
                rp_map[jq] = rp_ts

            def emit_taila(jq):
                # normalize this column block (broadcast 1/den over the 64
                # head rows via a K=1 matmul), ship to AllGather
                for p in range(2):
                    rp_t = rp_map[jq][p]
                    recb = pxp.tile([128, TQ], F32, tag="x", name="recb")
                    nc.tensor.matmul(
                        recb[:],
                        lhsT=bc2_sb[:],
                        rhs=rp_t[:],
                        start=True,
                        stop=True,
                    )
                    dst = oT[p][:, jq * TQ : (jq + 1) * TQ]
                    nc.vector.tensor_mul(out=dst, in0=dst, in1=recb[:])
                    nc.sync.dma_start(
                        out=ag_in[jq][p * 128 : (p + 1) * 128, :], in_=dst
                    )
                nc.gpsimd.collective_compute(
                    "AllGather",
                    mybir.AluOpType.bypass,
                    ins=[ag_in[jq][:]],
                    outs=[ag_out[jq][:]],
                    replica_groups=GROUPS,
                )

            def emit_tailb(jq):
                # gathered O^T back to SBUF, then the full-contraction
                # projection for this core's 256 output channels
                og = ogp.tile([128, NCK * TQ], ATT_DT, tag="og", name="og")
                nc.gpsimd.dma_start(
                    out=og.rearrange("p (k q) -> p k q", q=TQ),
                    in_=ag_out[jq].rearrange("(k p) q -> p k q", p=128),
                )
                for rt in range(2):
                    ps = pxp.tile([128, TQ], F32, tag="x", name="pmm")
                    for k in range(NCK):
                        nc.tensor.matmul(
                            ps[:],
                            lhsT=wp_sb[k][:, rt * 128 : (rt + 1) * 128],
                            rhs=og[:, k * TQ : (k + 1) * TQ],
                            start=(k == 0),
                            stop=(k == NCK - 1),
                        )
                    o = ocp.tile([128, TQ], F32, tag="oc", name="oc")
                    nc.vector.tensor_scalar_add(
                        out=o[:], in0=ps[:], scalar1=bp_sb[rt][:]
                    )
                    nc.sync.dma_start(
                        out=y_d[rt * 128 : (rt + 1) * 128, jq * TQ : (jq + 1) * TQ],
                        in_=o[:],
                    )

            for jq in range(NJQ):
                emit_qkv(jq)
                emit_attention(jq)
                emit_taila(jq)
                if jq >= 2:
                    emit_tailb(jq - 2)
            emit_tailb(NJQ - 2)
            emit_tailb(NJQ - 1)

    nc.compile()
    return nc


def _get_program():
    global _PROG
    if _PROG is None:
        _PROG = _build_program()
    return _PROG


def kernel(x, W_qkv, b_qkv, W_proj, b_proj):
    x = np.asarray(x, dtype=np.float32)
    W_qkv = np.asarray(W_qkv, dtype=np.float32)
    b_qkv = np.asarray(b_qkv, dtype=np.float32)
    W_proj = np.asarray(W_proj, dtype=np.float32)
    b_proj = np.asarray(b_proj, dtype=np.float32)

    nc = _get_program()

    mm_np = np.float16
    att_np = np.float16
    u = np.arange(TQ)[None, :]
    kl = np.arange(128)[:, None]
    m512 = (u >= kl).astype(att_np)
    mask_host = np.concatenate([m512, m512], axis=1)
    ones_host = np.ones((128, 64), dtype=np.float32)
    onesb_host = np.ones((128, 64), dtype=att_np)

    bc2_host = np.zeros((2, 128), dtype=np.float32)
    bc2_host[0, 0:64] = 1.0
    bc2_host[1, 64:128] = 1.0

    xts = [np.ascontiguousarray(x[b].T).astype(mm_np) for b in range(B)]
    in_maps = []
    for c in range(N_CORES):
        b, g = divmod(c, 4)
        cs = slice(g * G * DH, (g + 1) * G * DH)
        in_maps.append(
            {
                "xt": xts[b],
                "wq": np.ascontiguousarray(W_qkv[:, cs]).astype(mm_np),
                "wk": np.ascontiguousarray(W_qkv[:, C:][:, cs]).astype(mm_np),
                "wv": np.ascontiguousarray(W_qkv[:, 2 * C :][:, cs]).astype(mm_np),
                "wp": np.ascontiguousarray(W_proj[:, cs]).astype(mm_np),
                "bq": np.ascontiguousarray(b_qkv[cs]).reshape(-1, 1),
                "bk": np.ascontiguousarray(b_qkv[C:][cs]).reshape(-1, 1),
                "bv": np.ascontiguousarray(b_qkv[2 * C :][cs]).reshape(1, -1),
                "bp": np.ascontiguousarray(
                    b_proj[cs]
                ).reshape(-1, 1),
                "mask": mask_host,
                "bc2": bc2_host,
                "ones": ones_host,
                "onesb": onesb_host,
            }
        )

    global _last_in_maps
    _last_in_maps = in_maps
    res = run_bass_kernel_spmd(nc, in_maps, list(range(N_CORES)))

    y = np.empty((B, T, C), dtype=np.float32)
    for b in range(B):
        yT = np.concatenate(
            [res.results[4 * b + r]["y"] for r in range(4)], axis=0
        )
        y[b] = yT.T
    return y
